# revision 1
# baseline (speedup 1.0000x reference)
"""Trainium2 Bass kernel for nn_Dnn_with_Attention (ragged attention-pooled DNN).

Contract: kernel(**inputs) takes FULL unsharded numpy inputs (keys as in
reference.setup_inputs()) and returns the FULL [256, 10] float32 output.

Strategy (data-parallel over utterances, 8 NeuronCores):
  - Host: greedily balance the 256 segments over 8 cores (32 whole segments
    each), gather each core's frames, transpose x to feature-major
    [128(feat-padded), M_PAD] and build a per-frame one-hot segment
    membership matrix A [M_PAD/128, 128, 32].  A row of ones is appended as
    feature 78 so b1 folds into W1.
  - Device (per core): 4-layer MLP with activations kept feature-major
    (hT [1024, frames]) for layers 1-3; layer 4 produces frame-major
    h4 [128fr, 1024] (lhsT = h3T).  Scores via a DVE multiply + reduce
    against a replicated W5; e = exp(score) with the relu
    folded as max(e, 1).  Segment softmax pooling is done as small PE
    matmuls E.T @ h4 (E = A * e) accumulated into persistent PSUM across
    all chunks; the softmax denominator comes from E.T @ ones.  The final
    per-utterance MLP runs once at the end (W6 is DMA'd late into W4's
    SBUF slot to fit).
  - All matmuls use float32r (full-rate fp32 on the PE array); every
    matmul operand tile is typed float32r end-to-end to satisfy the
    walrus rounding rule.
"""

import sys

sys.path.insert(0, "/opt/trn_rl_repo")

import numpy as np

import concourse.bass as bass
import concourse.mybir as mybir
import concourse.tile as tile
from concourse import bacc
from concourse.bass_utils import run_bass_kernel_spmd

P = 128
FEAT = 78
HID = 1024
NCLS = 10
NSEG = 256
NCORES = 8
SEGS_PER_CORE = NSEG // NCORES
CH = 256           # frames per chunk (free dim of layer-1..3 matmuls)
FRT_PER_CH = CH // P
KS = HID // P      # 8 k-subtiles
F32 = mybir.dt.float32
F32R = mybir.dt.float32r

# misc constant tile column layout ([128, 256] f32, host-packed)
MC_B2 = 0          # cols 0..7   : b2 striped [128, 8]
MC_B3 = 8          # cols 8..15  : b3 striped
MC_B5 = 17         # col 17      : b5 replicated down partitions
MC_ID = 128        # cols 128..159, rows 0..31: 32x32 identity
# f32r matmul-constants tile ([128, 128])
MM_ONES = 0        # cols 0..7   : ones columns (denom matmul rhs, N=8)
MM_W7 = 16         # cols 16..95 : W7 as [128, 8, 10]
# row constants tile ([1, 192] f32r, host-packed)
RW_ONES = 0        # cols 0..127 : ones row
RW_B7 = 128        # cols 128..137 : b7


def _segment_ids(lengths: np.ndarray, total: int) -> np.ndarray:
    """Replicate jnp.repeat(arange(n), lengths, total_repeat_length=total)."""
    lengths = np.asarray(lengths, dtype=np.int64)
    seg = np.repeat(np.arange(lengths.shape[0], dtype=np.int32), np.maximum(lengths, 0))
    if seg.shape[0] >= total:
        return seg[:total]
    pad_val = seg[-1] if seg.shape[0] > 0 else np.int32(0)
    return np.concatenate([seg, np.full(total - seg.shape[0], pad_val, np.int32)])


def _balance_segments(lengths: np.ndarray) -> list[list[int]]:
    """Assign 256 segments to 8 cores, 32 each, minimizing max frame count."""
    order = np.argsort(-lengths, kind="stable")
    loads = [0] * NCORES
    bins: list[list[int]] = [[] for _ in range(NCORES)]
    for s in order:
        cands = [c for c in range(NCORES) if len(bins[c]) < SEGS_PER_CORE]
        c = min(cands, key=lambda c: (loads[c], c))
        bins[c].append(int(s))
        loads[c] += int(lengths[s])
    for b in bins:
        b.sort()
    return bins


UNROLL = 4         # chunks per hardware-loop iteration


def _build_program(m_pad: int):
    """Emit the Bass/Tile program for one core with m_pad frames (static).

    Chunks 0 and nch-1 are peeled (they carry the PSUM accumulation
    start/stop flags); the middle chunks run in a Tile hardware loop
    (For_i) so per-engine semaphore counts reset every back-edge and the
    instruction stream stays small.
    """
    nch = m_pad // CH
    frt = m_pad // P
    S = SEGS_PER_CORE

    nc = bacc.Bacc("TRN2", target_bir_lowering=False, debug=False,
                   num_devices=NCORES)

    xT_d = nc.dram_tensor("xT", [P, m_pad], F32R, kind="ExternalInput")
    A_d = nc.dram_tensor("Amat", [P, frt, S], F32, kind="ExternalInput")
    W1_d = nc.dram_tensor("W1p", [P, HID], F32R, kind="ExternalInput")
    W2_d = nc.dram_tensor("W2", [HID, HID], F32R, kind="ExternalInput")
    W3_d = nc.dram_tensor("W3", [HID, HID], F32R, kind="ExternalInput")
    W4_d = nc.dram_tensor("W4", [HID, HID], F32R, kind="ExternalInput")
    W5_d = nc.dram_tensor("W5rep", [P, HID], F32, kind="ExternalInput")
    W6_d = nc.dram_tensor("W6", [HID, HID], F32R, kind="ExternalInput")
    b4_d = nc.dram_tensor("b4r", [1, HID], F32R, kind="ExternalInput")
    b6_d = nc.dram_tensor("b6r", [1, HID], F32R, kind="ExternalInput")
    misc_d = nc.dram_tensor("miscc", [P, 256], F32, kind="ExternalInput")
    mmc_d = nc.dram_tensor("mmcc", [P, P], F32R, kind="ExternalInput")
    row_d = nc.dram_tensor("rowm", [1, 192], F32R, kind="ExternalInput")
    out_d = nc.dram_tensor("out", [S, NCLS], F32, kind="ExternalOutput")

    RELU = mybir.ActivationFunctionType.Relu
    EXP = mybir.ActivationFunctionType.Exp
    MULT = mybir.AluOpType.mult
    ADD = mybir.AluOpType.add

    with tile.TileContext(nc) as tc:
        with (
            tc.tile_pool(name="wpool", bufs=1) as wpool,
            tc.tile_pool(name="xpool", bufs=2) as xpool,
            tc.tile_pool(name="apool", bufs=2) as apool,
            tc.tile_pool(name="hpool", bufs=1) as hpool,
            tc.tile_pool(name="h4pool", bufs=2) as h4pool,
            tc.tile_pool(name="spool", bufs=1) as spool,
            tc.tile_pool(name="colpool", bufs=2) as colpool,
            tc.tile_pool(name="epool", bufs=2) as epool,
            tc.tile_pool(name="psA", bufs=3, space="PSUM") as psA,
            tc.tile_pool(name="psB", bufs=2, space="PSUM") as psB,
            tc.tile_pool(name="psAcc", bufs=1, space="PSUM") as psAcc,
        ):
            # ---- resident constants/weights ----
            W1s = wpool.tile([P, HID], F32R, tag="W1")
            nc.sync.dma_start(W1s[:], W1_d.ap())
            # per-k-subtile weight tiles: chunk-0 matmuls only wait on the
            # 0.5MB slice they read, not the whole 4MB matrix
            def load_wk(d, tagp):
                tiles = []
                for k in range(KS):
                    t = wpool.tile([P, HID], F32R, tag=f"{tagp}{k}")
                    nc.sync.dma_start(t[:], d.ap()[k * P:(k + 1) * P, :])
                    tiles.append(t)
                return tiles
            W2s = load_wk(W2_d, "W2k")
            W3s = load_wk(W3_d, "W3k")
            W4s = load_wk(W4_d, "W4k")
            W5s = wpool.tile([P, HID], F32, tag="W5")
            nc.sync.dma_start(W5s[:], W5_d.ap())
            b4s = wpool.tile([1, HID], F32R, tag="b4")
            nc.sync.dma_start(b4s[:], b4_d.ap())
            b6s = wpool.tile([1, HID], F32R, tag="b6")
            nc.sync.dma_start(b6s[:], b6_d.ap())
            misc = wpool.tile([P, 256], F32, tag="misc")
            nc.sync.dma_start(misc[:], misc_d.ap())
            mmc = wpool.tile([P, P], F32R, tag="mmc")
            nc.sync.dma_start(mmc[:], mmc_d.ap())
            rowm = wpool.tile([1, 192], F32R, tag="rowm")
            nc.sync.dma_start(rowm[:], row_d.ap())

            ones_row = rowm[:, RW_ONES:RW_ONES + P]
            ones_col = mmc[:, MM_ONES:MM_ONES + 8]
            b5col = misc[:, MC_B5:MC_B5 + 1]
            ident = misc[:S, MC_ID:MC_ID + S]
            W7v = mmc[:, MM_W7:MM_W7 + KS * NCLS].rearrange(
                "p (o c) -> p o c", c=NCLS)
            b7row = rowm[:, RW_B7:RW_B7 + NCLS]

            # persistent PSUM accumulators (own banks for the whole pass)
            pooled0 = psAcc.tile([S, 512], F32, tag="pooled0")
            pooled1 = psAcc.tile([S, 512], F32, tag="pooled1")
            denom = psAcc.tile([S, 8], F32, tag="denom")

            # ---- main pass over frame chunks ----
            def chunk_group(c0, n_chunks, first=False, last=False):
                """Emit n_chunks chunks starting at chunk index c0 (int or
                loop ScalarValue). first/last carry PSUM group flags."""
                xg = xpool.tile([P, UNROLL * CH], F32R, tag="x")
                nc.sync.dma_start(
                    xg[:, :n_chunks * CH],
                    xT_d.ap()[:, bass.ds(c0 * CH, n_chunks * CH)])
                ag = apool.tile([P, UNROLL * FRT_PER_CH, S], F32, tag="A")
                nc.sync.dma_start(
                    ag[:, :n_chunks * FRT_PER_CH, :],
                    A_d.ap()[:, bass.ds(c0 * FRT_PER_CH,
                                        n_chunks * FRT_PER_CH), :])

                for u in range(n_chunks):
                    xt = xg[:, u * CH:(u + 1) * CH]
                    # L1 (b1 folded via ones feature)
                    h1 = hpool.tile([P, KS, CH], F32R, tag="hA")
                    for m in range(KS):
                        ps = psA.tile([P, CH], F32, tag="mm")
                        nc.tensor.matmul(ps[:], W1s[:, m * P:(m + 1) * P], xt,
                                         start=True, stop=True)
                        nc.scalar.activation(h1[:, m, :], ps[:], RELU)

                    # L2 / L3 (h3 reuses h1's slot; h1 dead once L2 done)
                    h_in = h1
                    for Ws, boff, tag in ((W2s, MC_B2, "hB"), (W3s, MC_B3, "hA")):
                        h_out = hpool.tile([P, KS, CH], F32R, tag=tag)
                        for m in range(KS):
                            ps = psA.tile([P, CH], F32, tag="mm")
                            for k in range(KS):
                                nc.tensor.matmul(
                                    ps[:], Ws[k][:, m * P:(m + 1) * P],
                                    h_in[:, k, :],
                                    start=(k == 0), stop=(k == KS - 1))
                            nc.scalar.activation(
                                h_out[:, m, :], ps[:], RELU,
                                bias=misc[:, boff + m:boff + m + 1])
                        h_in = h_out
                    h3 = h_in

                    # L4 (frame-major) + scores + pooling per 128-frame tile
                    for f in range(FRT_PER_CH):
                        h4 = h4pool.tile([P, HID], F32R, tag="h4")
                        for n in range(2):
                            ps4 = psB.tile([P, 512], F32, tag="l4")
                            for k in range(KS):
                                nc.tensor.matmul(
                                    ps4[:], h3[:, k, f * P:(f + 1) * P],
                                    W4s[k][:, n * 512:(n + 1) * 512],
                                    start=(k == 0), stop=False)
                            nc.tensor.matmul(ps4[:], ones_row,
                                             b4s[:, n * 512:(n + 1) * 512],
                                             start=False, stop=True)
                            nc.scalar.activation(h4[:, n * 512:(n + 1) * 512],
                                                 ps4[:], RELU)

                        # scores: d = sum(h4*W5rep); e = max(exp(d + b5), 1)
                        prod = spool.tile([P, HID], F32, tag="sc")
                        ct = colpool.tile([P, 16], F32, tag="col")
                        nc.vector.tensor_tensor(
                            out=prod[:], in0=h4.bitcast(F32)[:], in1=W5s[:],
                            op=MULT)
                        nc.vector.tensor_reduce(
                            out=ct[:, 0:1], in_=prod[:],
                            axis=mybir.AxisListType.X, op=ADD)
                        nc.scalar.activation(ct[:, 1:2], ct[:, 0:1], EXP,
                                             bias=b5col)
                        nc.vector.tensor_scalar_max(ct[:, 2:3], ct[:, 1:2], 1.0)
                        et = epool.tile([P, S], F32R, tag="E")
                        nc.vector.tensor_scalar_mul(
                            et[:], ag[:, u * FRT_PER_CH + f, :], ct[:, 2:3])

                        st = bool(first and u == 0 and f == 0)
                        sp = bool(last and u == n_chunks - 1
                                  and f == FRT_PER_CH - 1)
                        nc.tensor.matmul(pooled0[:], et[:], h4[:, :512],
                                         start=st, stop=sp)
                        nc.tensor.matmul(pooled1[:], et[:], h4[:, 512:],
                                         start=st, stop=sp)
                        nc.tensor.matmul(denom[:], et[:], ones_col,
                                         start=st, stop=sp)

            # peel chunk 0 (PSUM group start) and chunk nch-1 (stop)
            import os
            chunk_group(0, 1, first=True)
            if os.environ.get("KERNEL_STATIC_UNROLL"):
                # cost-model twin: same stream, no dynamic loop machinery
                c = 1
                while c < nch - 1:
                    n = min(UNROLL, nch - 1 - c)
                    chunk_group(c, n)
                    c += n
            elif nch > 2:
                # 8 chunks per back-edge, emitted as 4-chunk DMA groups so
                # the x/A staging tiles stay at 4*CH
                def loop_body(iv, unroll):
                    off = 0
                    while off < unroll:
                        n = min(UNROLL, unroll - off)
                        chunk_group(iv + off, n)
                        off += n
                tc.For_i_unrolled_general(
                    start=1, end=nch - 1, step=1,
                    unrollable_body=loop_body,
                    max_unroll=2 * UNROLL,
                    hint_engines=(mybir.EngineType.PE,),
                )
            chunk_group(nch - 1, 1, last=True)

            # ---- final per-utterance MLP ----
            # W6 reuses W4's SBUF slots (W4 is dead after the last chunk)
            W6s = load_wk(W6_d, "W4k")

            fc = colpool.tile([S, 16], F32, tag="col")
            nc.vector.tensor_copy(out=fc[:, 0:1], in_=denom[:, 0:1])
            nc.vector.reciprocal(fc[:, 1:2], fc[:, 0:1])

            pooled_sb = spool.tile([S, HID], F32, tag="sc")
            nc.vector.tensor_scalar_mul(pooled_sb[:, :512], pooled0[:], fc[:, 1:2])
            nc.vector.tensor_scalar_mul(pooled_sb[:, 512:], pooled1[:], fc[:, 1:2])

            # transpose pooled -> pooledT [hid, seg]
            tposed = wpool.tile([P, KS, 2 * S], F32R, tag="tposed")
            pooledT = tposed[:, :, :S]
            gT = tposed[:, :, S:]
            for k in range(KS):
                pst = psA.tile([P, S], F32, tag="mm")
                nc.tensor.transpose(pst[:], pooled_sb[:, k * P:(k + 1) * P],
                                    ident)
                nc.vector.tensor_copy(out=pooledT[:, k, :], in_=pst[:])

            # g = relu(pooled @ W6 + b6)   (seg-major [S, HID])
            g_sb = spool.tile([S, HID], F32, tag="sc")
            for n in range(2):
                psg = psB.tile([S, 512], F32, tag="l4")
                for k in range(KS):
                    nc.tensor.matmul(psg[:], pooledT[:, k, :],
                                     W6s[k][:, n * 512:(n + 1) * 512],
                                     start=(k == 0), stop=False)
                nc.tensor.matmul(psg[:], ones_row[:, :S],
                                 b6s[:, n * 512:(n + 1) * 512],
                                 start=False, stop=True)
                nc.scalar.activation(g_sb[:, n * 512:(n + 1) * 512], psg[:], RELU)

            # gT [hid, seg]
            for k in range(KS):
                pst = psA.tile([P, S], F32, tag="mm")
                nc.tensor.transpose(pst[:], g_sb[:, k * P:(k + 1) * P], ident)
                nc.vector.tensor_copy(out=gT[:, k, :], in_=pst[:])

            # out = g @ W7 + b7
            pso = psA.tile([S, NCLS], F32, tag="mm")
            for k in range(KS):
                nc.tensor.matmul(pso[:], gT[:, k, :], W7v[:, k, :],
                                 start=(k == 0), stop=False)
            nc.tensor.matmul(pso[:], ones_row[:, :S], b7row,
                             start=False, stop=True)
            oc = colpool.tile([S, 16], F32, tag="col")
            nc.vector.tensor_copy(out=oc[:, :NCLS], in_=pso[:])
            nc.sync.dma_start(out_d.ap()[:], oc[:, :NCLS])

    nc.compile()
    return nc


def prepare_inputs(x, W1, b1, W2, b2, W3, b3, W4, b4, W5, b5, W6, b6, W7, b7,
                   lengths):
    """Host-side sharding/packing. Returns (in_maps, bins, m_pad)."""
    x = np.ascontiguousarray(np.asarray(x, dtype=np.float32))
    lengths = np.asarray(lengths)
    total = x.shape[0]
    seg_ids = _segment_ids(lengths, total)
    counts = np.bincount(seg_ids, minlength=NSEG).astype(np.int64)
    starts = np.zeros(NSEG + 1, dtype=np.int64)
    starts[1:] = np.cumsum(counts)

    bins = _balance_segments(counts)
    core_frames = [int(sum(counts[s] for s in b)) for b in bins]
    m_pad = ((max(core_frames) + CH - 1) // CH) * CH
    frt = m_pad // P

    W1p = np.zeros((P, HID), dtype=np.float32)
    W1p[:FEAT] = np.asarray(W1, dtype=np.float32)
    W1p[FEAT] = np.asarray(b1, dtype=np.float32)

    misc = np.zeros((P, 256), dtype=np.float32)
    misc[:, MC_B2:MC_B2 + KS] = np.asarray(b2, np.float32).reshape(KS, P).T
    misc[:, MC_B3:MC_B3 + KS] = np.asarray(b3, np.float32).reshape(KS, P).T
    misc[:, MC_B5] = np.float32(np.asarray(b5, np.float32).reshape(-1)[0])
    misc[:SEGS_PER_CORE, MC_ID:MC_ID + SEGS_PER_CORE] = np.eye(
        SEGS_PER_CORE, dtype=np.float32)

    mmcc = np.zeros((P, P), dtype=np.float32)
    mmcc[:, MM_ONES:MM_ONES + 8] = 1.0
    mmcc[:, MM_W7:MM_W7 + KS * NCLS] = np.asarray(W7, np.float32).reshape(
        KS, P, NCLS).transpose(1, 0, 2).reshape(P, KS * NCLS)

    rowm = np.zeros((1, 192), dtype=np.float32)
    rowm[0, RW_ONES:RW_ONES + P] = 1.0
    rowm[0, RW_B7:RW_B7 + NCLS] = np.asarray(b7, np.float32).reshape(-1)

    shared = dict(
        W1p=W1p,
        W2=np.ascontiguousarray(np.asarray(W2, np.float32)),
        W3=np.ascontiguousarray(np.asarray(W3, np.float32)),
        W4=np.ascontiguousarray(np.asarray(W4, np.float32)),
        W5rep=np.broadcast_to(np.asarray(W5, np.float32).reshape(1, HID),
                              (P, HID)).copy(),
        W6=np.ascontiguousarray(np.asarray(W6, np.float32)),
        b4r=np.asarray(b4, np.float32).reshape(1, HID),
        b6r=np.asarray(b6, np.float32).reshape(1, HID),
        miscc=misc,
        mmcc=mmcc,
        rowm=rowm,
    )

    in_maps = []
    for core in range(NCORES):
        segs = bins[core]
        xs = [x[starts[s]:starts[s + 1]] for s in segs]
        xcat = np.concatenate(xs, axis=0) if xs else np.zeros((0, FEAT), np.float32)
        n = xcat.shape[0]
        xT = np.zeros((P, m_pad), dtype=np.float32)
        xT[:FEAT, :n] = xcat.T
        xT[FEAT, :n] = 1.0  # constant feature -> b1
        A = np.zeros((m_pad, SEGS_PER_CORE), dtype=np.float32)
        off = 0
        for j, s in enumerate(segs):
            ln = int(counts[s])
            A[off:off + ln, j] = 1.0
            off += ln
        im = dict(shared)
        im["xT"] = xT
        # partition-major layout [P, frt, S]: Ah[p, t, s] = A[t*128 + p, s]
        im["Amat"] = np.ascontiguousarray(
            A.reshape(frt, P, SEGS_PER_CORE).transpose(1, 0, 2))
        in_maps.append(im)
    return in_maps, bins, m_pad


_PROGRAM_CACHE: dict[int, object] = {}


def kernel(**inputs) -> np.ndarray:
    in_maps, bins, m_pad = prepare_inputs(**inputs)
    nc = _PROGRAM_CACHE.get(m_pad)
    if nc is None:
        nc = _build_program(m_pad)
        _PROGRAM_CACHE[m_pad] = nc
    res = run_bass_kernel_spmd(nc, in_maps, core_ids=list(range(NCORES)))
    out = np.zeros((NSEG, NCLS), dtype=np.float32)
    for core in range(NCORES):
        out[bins[core]] = res.results[core]["out"]
    return out



# revision 7
# speedup vs baseline: 1.7848x; 1.7848x over previous
"""Trainium2 Bass kernel for nn_Dnn_with_Attention (ragged attention-pooled DNN).

Contract: kernel(**inputs) takes FULL unsharded numpy inputs (keys as in
reference.setup_inputs()) and returns the FULL [256, 10] float32 output.

Strategy (data-parallel over utterances, 8 NeuronCores):
  - Host: greedily balance the 256 segments over 8 cores (32 whole segments
    each), gather each core's frames, transpose x to feature-major
    bf16 [128(feat-padded), M_PAD] and build a per-frame one-hot segment
    membership matrix A (bf16).  A row of ones is appended as feature 78 so
    b1 folds into W1.
  - Device (per core): L1 in bf16 (feature-major, [1024, frames]); L2/L3/L4
    run in fp8 e4m3 with MatmulPerfMode.DoubleRow (two 128-K slices per
    instruction at 0.5 cycles/row, ~4x the f32r rate).  Weights W2/W3/W4 are
    host-quantized to e4m3; inter-layer activations are written as e4m3
    directly by the relu ops.  L4 produces frame-major h4 in bf16; b4 is
    added via a DoubleRow matmul against a host-packed (hi, lo) e4m3 pair so
    the quantization error cancels.  Scores use a single fused DVE
    tensor_tensor_reduce (h4 * W5 -> per-frame sum) in bf16 2x mode;
    e = max(exp(score + b5), 1) folds the relu.  Segment softmax pooling is
    small PE matmuls E.T @ h4 (E = A * e, bf16) accumulated into persistent
    PSUM across all chunks; the denominator comes from E.T @ ones into the
    same PSUM bank at a different partition quadrant.  The final
    per-utterance MLP runs once at the end in bf16.
  - Per-layer relu work is spread across three engines so the PE stays the
    bottleneck: L1 on GpSimd, L2/L4 on Scalar (activation), L3 on DVE
    (fused add+max tensor_scalar).
  - The whole program is emitted statically (no hardware loop).
"""

import sys

sys.path.insert(0, "/opt/trn_rl_repo")

import numpy as np
import ml_dtypes

import concourse.bass as bass
import concourse.mybir as mybir
import concourse.tile as tile
from concourse import bacc
from concourse.bass_utils import run_bass_kernel_spmd

P = 128
FEAT = 78
HID = 1024
NCLS = 10
NSEG = 256
NCORES = 8
SEGS_PER_CORE = NSEG // NCORES
CH = 512           # frames per chunk (free dim of the layer matmuls)
FRT_PER_CH = CH // P
KS = HID // P      # 8 k-subtiles
F32 = mybir.dt.float32
F32R = mybir.dt.float32r
BF16 = mybir.dt.bfloat16
F8 = mybir.dt.float8e4
DR = mybir.MatmulPerfMode.DoubleRow
E4NP = ml_dtypes.float8_e4m3
BFNP = ml_dtypes.bfloat16

# misc constant tile column layout ([128, 32] f32, host-packed)
MC_B2 = 0          # cols 0..7   : b2 striped [128, 8]
MC_B3 = 8          # cols 8..15  : b3 striped
MC_B5 = 17         # col 17      : b5 replicated down partitions
# bf16 const tile ([128, 96])
CB_ONES8 = 0       # cols 0..7  : ones (denom matmul rhs)
CB_ID = 8          # cols 8..39, rows 0..31: 32x32 identity
CB_W7 = 40         # cols 40..119?? keep within 96: W7 as [128, 8, 10] -> 80 cols
# fp8 const row ([1, 2, 1536]): ones pair + b4 (hi, lo) pair
# row layout [1, 2, 1536]: [:, :, 0:128] ones, [:, :, 512:1536] b4 hi/lo
# simpler: two fields side by side, see prepare_inputs
# bf16 row consts ([1, 64])
RW_B7 = 0          # cols 0..9 : b7
RW_ONES = 16       # cols 16..48 : ones row (bias matmuls, final MLP)


def _segment_ids(lengths: np.ndarray, total: int) -> np.ndarray:
    """Replicate jnp.repeat(arange(n), lengths, total_repeat_length=total)."""
    lengths = np.asarray(lengths, dtype=np.int64)
    seg = np.repeat(np.arange(lengths.shape[0], dtype=np.int32), np.maximum(lengths, 0))
    if seg.shape[0] >= total:
        return seg[:total]
    pad_val = seg[-1] if seg.shape[0] > 0 else np.int32(0)
    return np.concatenate([seg, np.full(total - seg.shape[0], pad_val, np.int32)])


def _balance_segments(lengths: np.ndarray) -> list[list[int]]:
    """Assign 256 segments to 8 cores, 32 each, minimizing max frame count."""
    order = np.argsort(-lengths, kind="stable")
    loads = [0] * NCORES
    bins: list[list[int]] = [[] for _ in range(NCORES)]
    for s in order:
        cands = [c for c in range(NCORES) if len(bins[c]) < SEGS_PER_CORE]
        c = min(cands, key=lambda c: (loads[c], c))
        bins[c].append(int(s))
        loads[c] += int(lengths[s])
    for b in bins:
        b.sort()
    return bins


def _build_program(m_pad: int):
    """Emit the Bass/Tile program for one core with m_pad frames (static)."""
    nch = m_pad // CH
    frt = m_pad // P
    S = SEGS_PER_CORE

    nc = bacc.Bacc("TRN2", target_bir_lowering=False, debug=False,
                   num_devices=NCORES)

    xT_d = nc.dram_tensor("xT", [P, m_pad], BF16, kind="ExternalInput")
    A_d = nc.dram_tensor("Amat", [P, frt, S], BF16, kind="ExternalInput")
    W1_d = nc.dram_tensor("W1p", [P, HID], BF16, kind="ExternalInput")
    W2_d = nc.dram_tensor("W2q", [P, KS, HID], F8, kind="ExternalInput")
    W3_d = nc.dram_tensor("W3q", [P, KS, HID], F8, kind="ExternalInput")
    W4_d = nc.dram_tensor("W4q", [P, KS, HID], F8, kind="ExternalInput")
    W5_d = nc.dram_tensor("W5rep", [P, HID], BF16, kind="ExternalInput")
    W6_d = nc.dram_tensor("W6b", [P, KS, HID], BF16, kind="ExternalInput")
    b6_d = nc.dram_tensor("b6r", [1, HID], BF16, kind="ExternalInput")
    misc_d = nc.dram_tensor("miscc", [P, 32], F32, kind="ExternalInput")
    cbf_d = nc.dram_tensor("cbf", [P, 128], BF16, kind="ExternalInput")
    c8_d = nc.dram_tensor("c8", [1, 2, 1536], F8, kind="ExternalInput")
    rw_d = nc.dram_tensor("rwb", [1, 64], BF16, kind="ExternalInput")
    out_d = nc.dram_tensor("out", [S, NCLS], F32, kind="ExternalOutput")

    RELU = mybir.ActivationFunctionType.Relu
    EXP = mybir.ActivationFunctionType.Exp
    MULT = mybir.AluOpType.mult
    ADD = mybir.AluOpType.add
    MAX = mybir.AluOpType.max

    with tile.TileContext(nc) as tc:
        with (
            tc.tile_pool(name="wpool", bufs=1) as wpool,
            tc.tile_pool(name="xpool", bufs=3) as xpool,
            tc.tile_pool(name="h1pool", bufs=2) as h1pool,
            tc.tile_pool(name="h2pool", bufs=2) as h2pool,
            tc.tile_pool(name="h3pool", bufs=2) as h3pool,
            tc.tile_pool(name="h4pool", bufs=2) as h4pool,
            tc.tile_pool(name="scrpool", bufs=2) as scrpool,
            tc.tile_pool(name="colpool", bufs=2) as colpool,
            tc.tile_pool(name="epool", bufs=2) as epool,
            tc.tile_pool(name="fpool", bufs=1) as fpool,
            tc.tile_pool(name="psL", bufs=3, space="PSUM") as psL,
            tc.tile_pool(name="psAcc", bufs=1, space="PSUM") as psAcc,
        ):
            # ---- resident constants/weights ----
            W1s = wpool.tile([P, HID], BF16, tag="W1")
            nc.sync.dma_start(W1s[:], W1_d.ap())

            def load_w(d, tagp, dt):
                t = wpool.tile([P, KS, HID], dt, tag=tagp)
                for k in range(KS):
                    nc.sync.dma_start(t[:, k, :], d.ap()[:, k, :])
                return t

            W2s = load_w(W2_d, "W2q", F8)
            W3s = load_w(W3_d, "W3q", F8)
            W4s = load_w(W4_d, "W4q", F8)
            W5s = wpool.tile([P, HID], BF16, tag="W5")
            nc.sync.dma_start(W5s[:], W5_d.ap())
            misc = wpool.tile([P, 32], F32, tag="misc")
            nc.sync.dma_start(misc[:], misc_d.ap())
            cbf = wpool.tile([P, 128], BF16, tag="cbf")
            nc.sync.dma_start(cbf[:], cbf_d.ap())
            c8 = wpool.tile([1, 2, 1536], F8, tag="c8")
            nc.sync.dma_start(c8[:], c8_d.ap())
            rwb = wpool.tile([1, 64], BF16, tag="rwb")
            nc.sync.dma_start(rwb[:], rw_d.ap())

            b5col = misc[:, MC_B5:MC_B5 + 1]
            ones8 = cbf[:, CB_ONES8:CB_ONES8 + 8]
            ident = cbf[:S, CB_ID:CB_ID + S]
            W7v = cbf[:, CB_W7:CB_W7 + KS * NCLS].rearrange(
                "p (o c) -> p o c", c=NCLS)
            b7row = rwb[:, RW_B7:RW_B7 + NCLS]
            ones_row = rwb[:, RW_ONES:RW_ONES + S]
            ones_pair8 = c8[:, :, 0:P]          # [1, 2, 128] of ones (fp8)
            b4pair = c8[:, :, 512:1536]         # [1, 2, 1024] b4 (hi, lo)

            # persistent PSUM accumulators:
            #   bank0: pooled0 [0:32, 0:512], denom [32:64, 0:8]
            #   bank1: pooled1 [0:32, 0:512]
            acc0 = psAcc.tile([P, 512], F32, tag="acc0")
            acc1 = psAcc.tile([P, 512], F32, tag="acc1")
            pooled0 = acc0[0:S, :]
            pooled1 = acc1[0:S, :]
            denom = acc0[S:2 * S, 0:8]

            # ---- main pass over frame chunks ----
            def chunk(c):
                first = c == 0
                last = c == nch - 1
                xt = xpool.tile([P, CH], BF16, tag="x")
                nc.sync.dma_start(xt[:], xT_d.ap()[:, c * CH:(c + 1) * CH])
                ag = xpool.tile([P, FRT_PER_CH, S], BF16, tag="A")
                nc.sync.dma_start(
                    ag[:], A_d.ap()[:, c * FRT_PER_CH:(c + 1) * FRT_PER_CH, :])

                # L1 (bf16, b1 folded via ones feature) -> h1 fp8.  m-tile
                # pairs share a 2-bank psum tile; one batched DVE relu each.
                h1 = h1pool.tile([P, KS, CH], F8, tag="h1")
                for j in range(KS // 2):
                    ps = psL.tile([P, 2 * CH], F32, tag="mm")
                    for i in range(2):
                        m = 2 * j + i
                        nc.tensor.matmul(ps[:, i * CH:(i + 1) * CH],
                                         W1s[:, m * P:(m + 1) * P], xt[:],
                                         start=True, stop=True)
                    nc.vector.tensor_scalar_max(h1[:, 2 * j:2 * j + 2, :],
                                                ps[:], 0.0)

                # L2 (fp8 DoubleRow) -> h2 fp8, Scalar relu(+b2) per m-tile
                h2 = h2pool.tile([P, KS, CH], F8, tag="h2")
                for j in range(KS // 2):
                    ps = psL.tile([P, 2 * CH], F32, tag="mm")
                    for i in range(2):
                        m = 2 * j + i
                        for t in range(KS // 2):
                            nc.tensor.matmul(
                                ps[:, i * CH:(i + 1) * CH],
                                W2s[:, 2 * t:2 * t + 2, m * P:(m + 1) * P],
                                h1[:, 2 * t:2 * t + 2, :],
                                start=(t == 0), stop=(t == KS // 2 - 1),
                                perf_mode=DR)
                        nc.scalar.activation(
                            h2[:, m, :], ps[:, i * CH:(i + 1) * CH], RELU,
                            bias=misc[:, MC_B2 + m:MC_B2 + m + 1])

                # L3 (fp8 DoubleRow) -> h3 fp8, DVE fused add+max per m-tile
                h3 = h3pool.tile([P, KS, CH], F8, tag="h3")
                for j in range(KS // 2):
                    ps = psL.tile([P, 2 * CH], F32, tag="mm")
                    for i in range(2):
                        m = 2 * j + i
                        for t in range(KS // 2):
                            nc.tensor.matmul(
                                ps[:, i * CH:(i + 1) * CH],
                                W3s[:, 2 * t:2 * t + 2, m * P:(m + 1) * P],
                                h2[:, 2 * t:2 * t + 2, :],
                                start=(t == 0), stop=(t == KS // 2 - 1),
                                perf_mode=DR)
                        nc.vector.tensor_scalar(
                            out=h3[:, m, :], in0=ps[:, i * CH:(i + 1) * CH],
                            scalar1=misc[:, MC_B3 + m:MC_B3 + m + 1],
                            scalar2=0.0, op0=ADD, op1=MAX)

                # L4 (fp8 DoubleRow, frame-major) -> h4 bf16, batched Scalar
                # relu.  b4 enters via a DoubleRow matmul on the (hi, lo) pair.
                h4 = h4pool.tile([P, FRT_PER_CH, HID], BF16, tag="h4")
                for f in range(FRT_PER_CH):
                    ps4 = psL.tile([P, 2 * CH], F32, tag="mm")
                    for n in range(2):
                        o = ps4[:, n * 512:(n + 1) * 512]
                        nc.tensor.matmul(o, ones_pair8,
                                         b4pair[:, :, n * 512:(n + 1) * 512],
                                         start=True, stop=False, perf_mode=DR)
                        for t in range(KS // 2):
                            nc.tensor.matmul(
                                o, h3[:, 2 * t:2 * t + 2, f * P:(f + 1) * P],
                                W4s[:, 2 * t:2 * t + 2, n * 512:(n + 1) * 512],
                                start=False, stop=(t == KS // 2 - 1),
                                perf_mode=DR)
                    nc.scalar.activation(h4[:, f, :], ps4[:], RELU)

                # scores + pooling per 128-frame tile.  Product on GpSimd
                # (SBUF only), free-axis reduce on DVE, exp on Scalar.
                for f in range(FRT_PER_CH):
                    scr = scrpool.tile([P, HID], BF16, tag="scr")
                    ct = colpool.tile([P, 4], F32, tag="col")
                    nc.gpsimd.tensor_mul(scr[:], h4[:, f, :], W5s[:])
                    nc.vector.tensor_reduce(out=ct[:, 0:1], in_=scr[:],
                                            axis=mybir.AxisListType.X, op=ADD)
                    nc.scalar.activation(ct[:, 1:2], ct[:, 0:1], EXP,
                                         bias=b5col)
                    nc.gpsimd.tensor_scalar_max(ct[:, 2:3], ct[:, 1:2], 1.0)
                    et = epool.tile([P, S], BF16, tag="E")
                    nc.gpsimd.tensor_scalar_mul(et[:], ag[:, f, :], ct[:, 2:3])

                    st = bool(first and f == 0)
                    sp = bool(last and f == FRT_PER_CH - 1)
                    # pooled0/denom share a PSUM bank at different partition
                    # quadrants; the sim's group check is partition-blind so
                    # it must be skipped (values verified exact in CoreSim).
                    nc.tensor.matmul(pooled0, et[:], h4[:, f, :512],
                                     start=st, stop=sp, skip_group_check=True)
                    nc.tensor.matmul(pooled1, et[:], h4[:, f, 512:],
                                     start=st, stop=sp, skip_group_check=True)
                    nc.tensor.matmul(denom, et[:], ones8,
                                     start=st, stop=sp, skip_group_check=True)

            for c in range(nch):
                chunk(c)

            # ---- final per-utterance MLP ----
            W6s = load_w(W6_d, "W6b", BF16)
            b6s = wpool.tile([1, HID], BF16, tag="b6")
            nc.sync.dma_start(b6s[:], b6_d.ap())

            # 1/denom: copy the [32:64] psum quadrant to SBUF, then DMA-shift
            # it down to partitions 0:32 (engines can't move across lanes)
            dtmp = fpool.tile([2 * S, 1], F32, tag="dtmp")
            nc.vector.tensor_copy(out=dtmp[S:2 * S, 0:1], in_=denom[:, 0:1])
            fc = colpool.tile([S, 4], F32, tag="col")
            nc.sync.dma_start(fc[:, 0:1], dtmp[S:2 * S, 0:1])
            nc.vector.reciprocal(fc[:, 1:2], fc[:, 0:1])

            # pooled (normalized) in f32 for the PE transpose
            pooled_sb = fpool.tile([S, HID], F32, tag="pooled")
            nc.vector.tensor_scalar_mul(pooled_sb[:, :512], pooled0,
                                        fc[:, 1:2])
            nc.vector.tensor_scalar_mul(pooled_sb[:, 512:], pooled1,
                                        fc[:, 1:2])

            # transpose pooled -> pooledT [hid, seg] (bf16 via cast copies)
            identf = fpool.tile([S, S], F32, tag="identf")
            nc.vector.tensor_copy(out=identf[:], in_=ident)
            tposed = fpool.tile([P, KS, 2 * S], BF16, tag="tposed")
            pooledT = tposed[:, :, :S]
            gT = tposed[:, :, S:]
            for k in range(KS):
                pst = psL.tile([P, 2 * CH], F32, tag="mm")
                nc.tensor.transpose(pst[:, :S], pooled_sb[:, k * P:(k + 1) * P],
                                    identf[:])
                nc.vector.tensor_copy(out=pooledT[:, k, :], in_=pst[:, :S])

            # g = relu(pooled @ W6 + b6)   (seg-major [S, HID], bf16)
            g_sb = fpool.tile([S, HID], BF16, tag="g")
            for n in range(2):
                psg = psL.tile([P, 2 * CH], F32, tag="mm")
                for k in range(KS):
                    nc.tensor.matmul(psg[:S, :512], pooledT[:, k, :],
                                     W6s[:, k, n * 512:(n + 1) * 512],
                                     start=(k == 0), stop=False)
                nc.tensor.matmul(psg[:S, :512], ones_row,
                                 b6s[:, n * 512:(n + 1) * 512],
                                 start=False, stop=True)
                nc.scalar.activation(g_sb[:, n * 512:(n + 1) * 512],
                                     psg[:S, :512], RELU)

            # gT [hid, seg] (transpose back via f32 staging)
            gf = fpool.tile([S, HID], F32, tag="gf")
            nc.vector.tensor_copy(out=gf[:], in_=g_sb[:])
            for k in range(KS):
                pst = psL.tile([P, 2 * CH], F32, tag="mm")
                nc.tensor.transpose(pst[:, :S], gf[:, k * P:(k + 1) * P],
                                    identf[:])
                nc.vector.tensor_copy(out=gT[:, k, :], in_=pst[:, :S])

            # out = g @ W7 + b7
            pso = psL.tile([P, 2 * CH], F32, tag="mm")
            for k in range(KS):
                nc.tensor.matmul(pso[:S, :NCLS], gT[:, k, :], W7v[:, k, :],
                                 start=(k == 0), stop=False)
            nc.tensor.matmul(pso[:S, :NCLS], ones_row, b7row,
                             start=False, stop=True)
            oc = colpool.tile([S, 16], F32, tag="oc")
            nc.vector.tensor_copy(out=oc[:, :NCLS], in_=pso[:S, :NCLS])
            nc.sync.dma_start(out_d.ap()[:], oc[:, :NCLS])

    nc.compile()
    return nc


def prepare_inputs(x, W1, b1, W2, b2, W3, b3, W4, b4, W5, b5, W6, b6, W7, b7,
                   lengths):
    """Host-side sharding/packing. Returns (in_maps, bins, m_pad)."""
    x = np.ascontiguousarray(np.asarray(x, dtype=np.float32))
    lengths = np.asarray(lengths)
    total = x.shape[0]
    seg_ids = _segment_ids(lengths, total)
    counts = np.bincount(seg_ids, minlength=NSEG).astype(np.int64)
    starts = np.zeros(NSEG + 1, dtype=np.int64)
    starts[1:] = np.cumsum(counts)

    bins = _balance_segments(counts)
    core_frames = [int(sum(counts[s] for s in b)) for b in bins]
    m_pad = ((max(core_frames) + CH - 1) // CH) * CH
    frt = m_pad // P

    W1p = np.zeros((P, HID), dtype=np.float32)
    W1p[:FEAT] = np.asarray(W1, dtype=np.float32)
    W1p[FEAT] = np.asarray(b1, dtype=np.float32)

    def dr_pack(W, dt):
        """[1024, 1024] -> [128, 8, 1024] with Wq[p, k, m] = W[k*128+p, m]."""
        Wf = np.asarray(W, np.float32).reshape(KS, P, HID)
        return np.ascontiguousarray(Wf.transpose(1, 0, 2)).astype(dt)

    misc = np.zeros((P, 32), dtype=np.float32)
    misc[:, MC_B2:MC_B2 + KS] = np.asarray(b2, np.float32).reshape(KS, P).T
    misc[:, MC_B3:MC_B3 + KS] = np.asarray(b3, np.float32).reshape(KS, P).T
    misc[:, MC_B5] = np.float32(np.asarray(b5, np.float32).reshape(-1)[0])

    cbf = np.zeros((P, 128), dtype=np.float32)
    cbf[:, CB_ONES8:CB_ONES8 + 8] = 1.0
    cbf[:SEGS_PER_CORE, CB_ID:CB_ID + SEGS_PER_CORE] = np.eye(
        SEGS_PER_CORE, dtype=np.float32)
    cbf[:, CB_W7:CB_W7 + KS * NCLS] = np.asarray(W7, np.float32).reshape(
        KS, P, NCLS).transpose(1, 0, 2).reshape(P, KS * NCLS)

    rwb = np.zeros((1, 64), dtype=np.float32)
    rwb[0, RW_B7:RW_B7 + NCLS] = np.asarray(b7, np.float32).reshape(-1)
    rwb[0, RW_ONES:RW_ONES + SEGS_PER_CORE] = 1.0

    c8 = np.zeros((1, 2, 1536), dtype=np.float32)
    c8[0, :, 0:P] = 1.0
    b4f = np.asarray(b4, np.float32).reshape(-1)
    b4hi = b4f.astype(E4NP).astype(np.float32)
    b4lo = (b4f - b4hi).astype(E4NP).astype(np.float32)
    c8[0, 0, 512:1536] = b4hi
    c8[0, 1, 512:1536] = b4lo

    shared = dict(
        W1p=W1p.astype(BFNP),
        W2q=dr_pack(W2, E4NP),
        W3q=dr_pack(W3, E4NP),
        W4q=dr_pack(W4, E4NP),
        W5rep=np.broadcast_to(np.asarray(W5, np.float32).reshape(1, HID),
                              (P, HID)).astype(BFNP),
        W6b=dr_pack(W6, BFNP),
        b6r=np.asarray(b6, np.float32).reshape(1, HID).astype(BFNP),
        miscc=misc,
        cbf=cbf.astype(BFNP),
        c8=c8.astype(E4NP),
        rwb=rwb.astype(BFNP),
    )

    in_maps = []
    for core in range(NCORES):
        segs = bins[core]
        xs = [x[starts[s]:starts[s + 1]] for s in segs]
        xcat = np.concatenate(xs, axis=0) if xs else np.zeros((0, FEAT), np.float32)
        n = xcat.shape[0]
        xT = np.zeros((P, m_pad), dtype=np.float32)
        xT[:FEAT, :n] = xcat.T
        xT[FEAT, :n] = 1.0  # constant feature -> b1
        A = np.zeros((m_pad, SEGS_PER_CORE), dtype=np.float32)
        off = 0
        for j, s in enumerate(segs):
            ln = int(counts[s])
            A[off:off + ln, j] = 1.0
            off += ln
        im = dict(shared)
        im["xT"] = xT.astype(BFNP)
        # partition-major layout [P, frt, S]: Ah[p, t, s] = A[t*128 + p, s]
        im["Amat"] = np.ascontiguousarray(
            A.reshape(frt, P, SEGS_PER_CORE).transpose(1, 0, 2)).astype(BFNP)
        in_maps.append(im)
    return in_maps, bins, m_pad


_PROGRAM_CACHE: dict[int, object] = {}


def kernel(**inputs) -> np.ndarray:
    in_maps, bins, m_pad = prepare_inputs(**inputs)
    nc = _PROGRAM_CACHE.get(m_pad)
    if nc is None:
        nc = _build_program(m_pad)
        _PROGRAM_CACHE[m_pad] = nc
    res = run_bass_kernel_spmd(nc, in_maps, core_ids=list(range(NCORES)))
    out = np.zeros((NSEG, NCLS), dtype=np.float32)
    for core in range(NCORES):
        out[bins[core]] = res.results[core]["out"]
    return out


# revision 9
# speedup vs baseline: 2.0295x; 1.1371x over previous
"""Trainium2 Bass kernel for nn_Dnn_with_Attention (ragged attention-pooled DNN).

Contract: kernel(**inputs) takes FULL unsharded numpy inputs (keys as in
reference.setup_inputs()) and returns the FULL [256, 10] float32 output.

Strategy (data-parallel over utterances, 8 NeuronCores):
  - Host: greedily balance the 256 segments over 8 cores (32 whole segments
    each), gather each core's frames, transpose x to feature-major
    bf16 [128(feat-padded), M_PAD] and build a per-frame one-hot segment
    membership matrix A (bf16).  A row of ones is appended as feature 78 so
    b1 folds into W1.
  - Device (per core): L1 in bf16 (feature-major, [1024, frames]); L2/L3/L4
    run in fp8 e4m3 with MatmulPerfMode.DoubleRow (two 128-K slices per
    instruction at 0.5 cycles/row, ~4x the f32r rate).  Weights W2/W3/W4 are
    host-quantized to e4m3; inter-layer activations are written as e4m3
    directly by the relu ops.  L4 produces frame-major h4 in bf16; b4 is
    added via a DoubleRow matmul against a host-packed (hi, lo) e4m3 pair so
    the quantization error cancels.  Scores use a single fused DVE
    tensor_tensor_reduce (h4 * W5 -> per-frame sum) in bf16 2x mode;
    e = max(exp(score + b5), 1) folds the relu.  Segment softmax pooling is
    small PE matmuls E.T @ h4 (E = A * e, bf16) accumulated into persistent
    PSUM across all chunks; the denominator comes from E.T @ ones into the
    same PSUM bank at a different partition quadrant.  The final
    per-utterance MLP runs once at the end in bf16.
  - Per-layer relu work is spread across three engines so the PE stays the
    bottleneck: L1 on GpSimd, L2/L4 on Scalar (activation), L3 on DVE
    (fused add+max tensor_scalar).
  - The whole program is emitted statically (no hardware loop).
"""

import sys

sys.path.insert(0, "/opt/trn_rl_repo")

import numpy as np
import ml_dtypes

import concourse.bass as bass
import concourse.mybir as mybir
import concourse.tile as tile
from concourse import bacc
from concourse.bass_utils import run_bass_kernel_spmd

P = 128
FEAT = 78
HID = 1024
NCLS = 10
NSEG = 256
NCORES = 8
SEGS_PER_CORE = NSEG // NCORES
CH = 512           # frames per chunk (free dim of the layer matmuls)
FRT_PER_CH = CH // P
KS = HID // P      # 8 k-subtiles
F32 = mybir.dt.float32
F32R = mybir.dt.float32r
BF16 = mybir.dt.bfloat16
F8 = mybir.dt.float8e4
DR = mybir.MatmulPerfMode.DoubleRow
E4NP = ml_dtypes.float8_e4m3
BFNP = ml_dtypes.bfloat16

# misc constant tile column layout ([128, 32] f32, host-packed)
MC_B2 = 0          # cols 0..7   : b2 striped [128, 8]
MC_B3 = 8          # cols 8..15  : b3 striped
MC_B5 = 17         # col 17      : b5 replicated down partitions
# bf16 const tile ([128, 96])
CB_ONES8 = 0       # cols 0..7  : ones (denom matmul rhs)
CB_ID = 8          # cols 8..39, rows 0..31: 32x32 identity
CB_W7 = 40         # cols 40..119?? keep within 96: W7 as [128, 8, 10] -> 80 cols
# fp8 const row ([1, 2, 1536]): ones pair + b4 (hi, lo) pair
# row layout [1, 2, 1536]: [:, :, 0:128] ones, [:, :, 512:1536] b4 hi/lo
# simpler: two fields side by side, see prepare_inputs
# bf16 row consts ([1, 64])
RW_B7 = 0          # cols 0..9 : b7
RW_ONES = 16       # cols 16..48 : ones row (bias matmuls, final MLP)


def _segment_ids(lengths: np.ndarray, total: int) -> np.ndarray:
    """Replicate jnp.repeat(arange(n), lengths, total_repeat_length=total)."""
    lengths = np.asarray(lengths, dtype=np.int64)
    seg = np.repeat(np.arange(lengths.shape[0], dtype=np.int32), np.maximum(lengths, 0))
    if seg.shape[0] >= total:
        return seg[:total]
    pad_val = seg[-1] if seg.shape[0] > 0 else np.int32(0)
    return np.concatenate([seg, np.full(total - seg.shape[0], pad_val, np.int32)])


def _balance_segments(lengths: np.ndarray) -> list[list[int]]:
    """Assign 256 segments to 8 cores, 32 each, minimizing max frame count."""
    order = np.argsort(-lengths, kind="stable")
    loads = [0] * NCORES
    bins: list[list[int]] = [[] for _ in range(NCORES)]
    for s in order:
        cands = [c for c in range(NCORES) if len(bins[c]) < SEGS_PER_CORE]
        c = min(cands, key=lambda c: (loads[c], c))
        bins[c].append(int(s))
        loads[c] += int(lengths[s])
    for b in bins:
        b.sort()
    return bins


def _build_program(m_pad: int):
    """Emit the Bass/Tile program for one core with m_pad frames (static)."""
    nch = m_pad // CH
    frt = m_pad // P
    S = SEGS_PER_CORE

    nc = bacc.Bacc("TRN2", target_bir_lowering=False, debug=False,
                   num_devices=NCORES)

    xT_d = nc.dram_tensor("xT", [P, m_pad], BF16, kind="ExternalInput")
    A_d = nc.dram_tensor("Amat", [P, frt, S], BF16, kind="ExternalInput")
    W1_d = nc.dram_tensor("W1p", [P, HID], BF16, kind="ExternalInput")
    W2_d = nc.dram_tensor("W2q", [P, KS, HID], F8, kind="ExternalInput")
    W3_d = nc.dram_tensor("W3q", [P, KS, HID], F8, kind="ExternalInput")
    W4_d = nc.dram_tensor("W4q", [P, KS, HID], F8, kind="ExternalInput")
    W5_d = nc.dram_tensor("W5rep", [P, HID], BF16, kind="ExternalInput")
    W6_d = nc.dram_tensor("W6b", [P, KS, HID], BF16, kind="ExternalInput")
    b6_d = nc.dram_tensor("b6r", [1, HID], BF16, kind="ExternalInput")
    misc_d = nc.dram_tensor("miscc", [P, 32], F32, kind="ExternalInput")
    cbf_d = nc.dram_tensor("cbf", [P, 128], BF16, kind="ExternalInput")
    c8_d = nc.dram_tensor("c8", [1, 2, 1536], F8, kind="ExternalInput")
    rw_d = nc.dram_tensor("rwb", [1, 64], BF16, kind="ExternalInput")
    out_d = nc.dram_tensor("out", [S, NCLS], F32, kind="ExternalOutput")

    RELU = mybir.ActivationFunctionType.Relu
    EXP = mybir.ActivationFunctionType.Exp
    MULT = mybir.AluOpType.mult
    ADD = mybir.AluOpType.add
    MAX = mybir.AluOpType.max

    with tile.TileContext(nc) as tc:
        with (
            tc.tile_pool(name="wpool", bufs=1) as wpool,
            tc.tile_pool(name="xpool", bufs=2) as xpool,
            tc.tile_pool(name="apool", bufs=5) as apool,
            tc.tile_pool(name="h1pool", bufs=2) as h1pool,
            tc.tile_pool(name="h2pool", bufs=2) as h2pool,
            tc.tile_pool(name="h3pool", bufs=2) as h3pool,
            tc.tile_pool(name="h4pool", bufs=2) as h4pool,
            tc.tile_pool(name="scrpool", bufs=2) as scrpool,
            tc.tile_pool(name="colpool", bufs=2) as colpool,
            tc.tile_pool(name="epool", bufs=2) as epool,
            tc.tile_pool(name="fpool", bufs=1) as fpool,
            tc.tile_pool(name="psL", bufs=3, space="PSUM") as psL,
            tc.tile_pool(name="psAcc", bufs=1, space="PSUM") as psAcc,
        ):
            # ---- resident constants/weights ----
            W1s = wpool.tile([P, HID], BF16, tag="W1")
            nc.sync.dma_start(W1s[:], W1_d.ap())

            def load_w(d, tagp, dt):
                t = wpool.tile([P, KS, HID], dt, tag=tagp)
                for k in range(KS):
                    nc.sync.dma_start(t[:, k, :], d.ap()[:, k, :])
                return t

            W2s = load_w(W2_d, "W2q", F8)
            W3s = load_w(W3_d, "W3q", F8)
            W4s = load_w(W4_d, "W4q", F8)
            W5s = wpool.tile([P, HID], BF16, tag="W5")
            nc.sync.dma_start(W5s[:], W5_d.ap())
            misc = wpool.tile([P, 32], F32, tag="misc")
            nc.sync.dma_start(misc[:], misc_d.ap())
            cbf = wpool.tile([P, 128], BF16, tag="cbf")
            nc.sync.dma_start(cbf[:], cbf_d.ap())
            c8 = wpool.tile([1, 2, 1536], F8, tag="c8")
            nc.sync.dma_start(c8[:], c8_d.ap())
            rwb = wpool.tile([1, 64], BF16, tag="rwb")
            nc.sync.dma_start(rwb[:], rw_d.ap())

            b5col = misc[:, MC_B5:MC_B5 + 1]
            ones8 = cbf[:, CB_ONES8:CB_ONES8 + 8]
            ident = cbf[:S, CB_ID:CB_ID + S]
            W7v = cbf[:, CB_W7:CB_W7 + KS * NCLS].rearrange(
                "p (o c) -> p o c", c=NCLS)
            b7row = rwb[:, RW_B7:RW_B7 + NCLS]
            ones_row = rwb[:, RW_ONES:RW_ONES + S]
            ones_pair8 = c8[:, :, 0:P]          # [1, 2, 128] of ones (fp8)
            b4pair = c8[:, :, 512:1536]         # [1, 2, 1024] b4 (hi, lo)

            # persistent PSUM accumulators:
            #   bank0: pooled0 [0:32, 0:512], denom [32:64, 0:8]
            #   bank1: pooled1 [0:32, 0:512]
            acc0 = psAcc.tile([P, 512], F32, tag="acc0")
            acc1 = psAcc.tile([P, 512], F32, tag="acc1")
            pooled0 = acc0[0:S, :]
            pooled1 = acc1[0:S, :]
            denom = acc0[S:2 * S, 0:8]

            # ---- main pass: 5-stage software pipeline over chunks ----
            # Stage k of chunk c runs in iteration c+k, so every cross-engine
            # dependency (matmul -> relu -> next layer's matmul) has a full
            # iteration (~15us) of slack and the PE never waits on the relus.
            st_ = {}   # per-chunk tile state

            def s1(c):  # DMA + L1 (bf16) -> h1 fp8 (DVE batched relu)
                xt = xpool.tile([P, CH], BF16, tag="x")
                nc.sync.dma_start(xt[:], xT_d.ap()[:, c * CH:(c + 1) * CH])
                ag = apool.tile([P, FRT_PER_CH, S], BF16, tag="A")
                nc.sync.dma_start(
                    ag[:], A_d.ap()[:, c * FRT_PER_CH:(c + 1) * FRT_PER_CH, :])
                h1 = h1pool.tile([P, KS, CH], F8, tag="h1")
                for j in range(KS // 2):
                    ps = psL.tile([P, 2 * CH], F32, tag="mm")
                    for i in range(2):
                        m = 2 * j + i
                        nc.tensor.matmul(ps[:, i * CH:(i + 1) * CH],
                                         W1s[:, m * P:(m + 1) * P], xt[:],
                                         start=True, stop=True)
                    nc.vector.tensor_scalar_max(h1[:, 2 * j:2 * j + 2, :],
                                                ps[:], 0.0)
                st_[c] = {"ag": ag, "h1": h1}

            def s2(c):  # L2 fp8 DoubleRow -> h2 fp8 (Scalar relu + b2)
                h1 = st_[c]["h1"]
                h2 = h2pool.tile([P, KS, CH], F8, tag="h2")
                for j in range(KS // 2):
                    ps = psL.tile([P, 2 * CH], F32, tag="mm")
                    for i in range(2):
                        m = 2 * j + i
                        for t in range(KS // 2):
                            nc.tensor.matmul(
                                ps[:, i * CH:(i + 1) * CH],
                                W2s[:, 2 * t:2 * t + 2, m * P:(m + 1) * P],
                                h1[:, 2 * t:2 * t + 2, :],
                                start=(t == 0), stop=(t == KS // 2 - 1),
                                perf_mode=DR)
                        nc.scalar.activation(
                            h2[:, m, :], ps[:, i * CH:(i + 1) * CH], RELU,
                            bias=misc[:, MC_B2 + m:MC_B2 + m + 1])
                st_[c]["h2"] = h2

            def s3(c):  # L3 fp8 DoubleRow -> h3 fp8 (DVE fused add+max)
                h2 = st_[c]["h2"]
                h3 = h3pool.tile([P, KS, CH], F8, tag="h3")
                for j in range(KS // 2):
                    ps = psL.tile([P, 2 * CH], F32, tag="mm")
                    for i in range(2):
                        m = 2 * j + i
                        for t in range(KS // 2):
                            nc.tensor.matmul(
                                ps[:, i * CH:(i + 1) * CH],
                                W3s[:, 2 * t:2 * t + 2, m * P:(m + 1) * P],
                                h2[:, 2 * t:2 * t + 2, :],
                                start=(t == 0), stop=(t == KS // 2 - 1),
                                perf_mode=DR)
                        nc.vector.tensor_scalar(
                            out=h3[:, m, :], in0=ps[:, i * CH:(i + 1) * CH],
                            scalar1=misc[:, MC_B3 + m:MC_B3 + m + 1],
                            scalar2=0.0, op0=ADD, op1=MAX)
                st_[c]["h3"] = h3

            def s4(c):  # L4 fp8 DoubleRow -> h4 bf16; scores -> E (bf16)
                h3 = st_[c]["h3"]
                ag = st_[c]["ag"]
                h4 = h4pool.tile([P, FRT_PER_CH, HID], BF16, tag="h4")
                for f in range(FRT_PER_CH):
                    ps4 = psL.tile([P, 2 * CH], F32, tag="mm")
                    for n in range(2):
                        o = ps4[:, n * 512:(n + 1) * 512]
                        nc.tensor.matmul(o, ones_pair8,
                                         b4pair[:, :, n * 512:(n + 1) * 512],
                                         start=True, stop=False, perf_mode=DR)
                        for t in range(KS // 2):
                            nc.tensor.matmul(
                                o, h3[:, 2 * t:2 * t + 2, f * P:(f + 1) * P],
                                W4s[:, 2 * t:2 * t + 2, n * 512:(n + 1) * 512],
                                start=False, stop=(t == KS // 2 - 1),
                                perf_mode=DR)
                    nc.scalar.activation(h4[:, f, :], ps4[:], RELU)
                # scores: product on GpSimd (SBUF only), free-axis reduce on
                # DVE, exp on Scalar, clamp/E on GpSimd.  E is consumed by
                # the pooling matmuls one iteration later.
                etg = epool.tile([P, FRT_PER_CH, S], BF16, tag="E")
                ct = colpool.tile([P, 16], F32, tag="col")
                for f in range(FRT_PER_CH):
                    scr = scrpool.tile([P, HID], BF16, tag="scr")
                    nc.gpsimd.tensor_mul(scr[:], h4[:, f, :], W5s[:])
                    nc.vector.tensor_reduce(out=ct[:, 4 * f:4 * f + 1],
                                            in_=scr[:],
                                            axis=mybir.AxisListType.X, op=ADD)
                    nc.scalar.activation(ct[:, 4 * f + 1:4 * f + 2],
                                         ct[:, 4 * f:4 * f + 1], EXP,
                                         bias=b5col)
                    nc.gpsimd.tensor_scalar_max(ct[:, 4 * f + 2:4 * f + 3],
                                                ct[:, 4 * f + 1:4 * f + 2],
                                                1.0)
                    nc.gpsimd.tensor_scalar_mul(etg[:, f, :], ag[:, f, :],
                                                ct[:, 4 * f + 2:4 * f + 3])
                st_[c]["h4"] = h4
                st_[c]["et"] = etg

            def s5(c):  # pooling matmuls (persistent PSUM accumulation)
                h4 = st_[c]["h4"]
                etg = st_[c]["et"]
                first = c == 0
                last = c == nch - 1
                for f in range(FRT_PER_CH):
                    et = etg[:, f, :]
                    st = bool(first and f == 0)
                    sp = bool(last and f == FRT_PER_CH - 1)
                    # pooled0/denom share a PSUM bank at different partition
                    # quadrants; the sim's group check is partition-blind so
                    # it must be skipped (values verified exact in CoreSim).
                    nc.tensor.matmul(pooled0, et, h4[:, f, :512],
                                     start=st, stop=sp, skip_group_check=True)
                    nc.tensor.matmul(pooled1, et, h4[:, f, 512:],
                                     start=st, stop=sp, skip_group_check=True)
                    nc.tensor.matmul(denom, et, ones8,
                                     start=st, stop=sp, skip_group_check=True)
                del st_[c]

            stages = (s1, s2, s3, s4, s5)
            for i in range(nch + len(stages) - 1):
                for k, stage in enumerate(stages):
                    c = i - k
                    if 0 <= c < nch:
                        stage(c)

            # ---- final per-utterance MLP ----
            W6s = load_w(W6_d, "W6b", BF16)
            b6s = wpool.tile([1, HID], BF16, tag="b6")
            nc.sync.dma_start(b6s[:], b6_d.ap())

            # 1/denom: copy the [32:64] psum quadrant to SBUF, then DMA-shift
            # it down to partitions 0:32 (engines can't move across lanes)
            dtmp = fpool.tile([2 * S, 1], F32, tag="dtmp")
            nc.vector.tensor_copy(out=dtmp[S:2 * S, 0:1], in_=denom[:, 0:1])
            fc = colpool.tile([S, 4], F32, tag="col")
            nc.sync.dma_start(fc[:, 0:1], dtmp[S:2 * S, 0:1])
            nc.vector.reciprocal(fc[:, 1:2], fc[:, 0:1])

            # pooled (normalized) in f32 for the PE transpose
            pooled_sb = fpool.tile([S, HID], F32, tag="pooled")
            nc.vector.tensor_scalar_mul(pooled_sb[:, :512], pooled0,
                                        fc[:, 1:2])
            nc.vector.tensor_scalar_mul(pooled_sb[:, 512:], pooled1,
                                        fc[:, 1:2])

            # transpose pooled -> pooledT [hid, seg] (bf16 via cast copies)
            identf = fpool.tile([S, S], F32, tag="identf")
            nc.vector.tensor_copy(out=identf[:], in_=ident)
            tposed = fpool.tile([P, KS, 2 * S], BF16, tag="tposed")
            pooledT = tposed[:, :, :S]
            gT = tposed[:, :, S:]
            for k in range(KS):
                pst = psL.tile([P, 2 * CH], F32, tag="mm")
                nc.tensor.transpose(pst[:, :S], pooled_sb[:, k * P:(k + 1) * P],
                                    identf[:])
                nc.vector.tensor_copy(out=pooledT[:, k, :], in_=pst[:, :S])

            # g = relu(pooled @ W6 + b6)   (seg-major [S, HID], bf16)
            g_sb = fpool.tile([S, HID], BF16, tag="g")
            for n in range(2):
                psg = psL.tile([P, 2 * CH], F32, tag="mm")
                for k in range(KS):
                    nc.tensor.matmul(psg[:S, :512], pooledT[:, k, :],
                                     W6s[:, k, n * 512:(n + 1) * 512],
                                     start=(k == 0), stop=False)
                nc.tensor.matmul(psg[:S, :512], ones_row,
                                 b6s[:, n * 512:(n + 1) * 512],
                                 start=False, stop=True)
                nc.scalar.activation(g_sb[:, n * 512:(n + 1) * 512],
                                     psg[:S, :512], RELU)

            # gT [hid, seg] (transpose back via f32 staging)
            gf = fpool.tile([S, HID], F32, tag="gf")
            nc.vector.tensor_copy(out=gf[:], in_=g_sb[:])
            for k in range(KS):
                pst = psL.tile([P, 2 * CH], F32, tag="mm")
                nc.tensor.transpose(pst[:, :S], gf[:, k * P:(k + 1) * P],
                                    identf[:])
                nc.vector.tensor_copy(out=gT[:, k, :], in_=pst[:, :S])

            # out = g @ W7 + b7
            pso = psL.tile([P, 2 * CH], F32, tag="mm")
            for k in range(KS):
                nc.tensor.matmul(pso[:S, :NCLS], gT[:, k, :], W7v[:, k, :],
                                 start=(k == 0), stop=False)
            nc.tensor.matmul(pso[:S, :NCLS], ones_row, b7row,
                             start=False, stop=True)
            oc = colpool.tile([S, 16], F32, tag="oc")
            nc.vector.tensor_copy(out=oc[:, :NCLS], in_=pso[:S, :NCLS])
            nc.sync.dma_start(out_d.ap()[:], oc[:, :NCLS])

    nc.compile()
    return nc


def prepare_inputs(x, W1, b1, W2, b2, W3, b3, W4, b4, W5, b5, W6, b6, W7, b7,
                   lengths):
    """Host-side sharding/packing. Returns (in_maps, bins, m_pad)."""
    x = np.ascontiguousarray(np.asarray(x, dtype=np.float32))
    lengths = np.asarray(lengths)
    total = x.shape[0]
    seg_ids = _segment_ids(lengths, total)
    counts = np.bincount(seg_ids, minlength=NSEG).astype(np.int64)
    starts = np.zeros(NSEG + 1, dtype=np.int64)
    starts[1:] = np.cumsum(counts)

    bins = _balance_segments(counts)
    core_frames = [int(sum(counts[s] for s in b)) for b in bins]
    m_pad = ((max(core_frames) + CH - 1) // CH) * CH
    frt = m_pad // P

    W1p = np.zeros((P, HID), dtype=np.float32)
    W1p[:FEAT] = np.asarray(W1, dtype=np.float32)
    W1p[FEAT] = np.asarray(b1, dtype=np.float32)

    def dr_pack(W, dt):
        """[1024, 1024] -> [128, 8, 1024] with Wq[p, k, m] = W[k*128+p, m]."""
        Wf = np.asarray(W, np.float32).reshape(KS, P, HID)
        return np.ascontiguousarray(Wf.transpose(1, 0, 2)).astype(dt)

    misc = np.zeros((P, 32), dtype=np.float32)
    misc[:, MC_B2:MC_B2 + KS] = np.asarray(b2, np.float32).reshape(KS, P).T
    misc[:, MC_B3:MC_B3 + KS] = np.asarray(b3, np.float32).reshape(KS, P).T
    misc[:, MC_B5] = np.float32(np.asarray(b5, np.float32).reshape(-1)[0])

    cbf = np.zeros((P, 128), dtype=np.float32)
    cbf[:, CB_ONES8:CB_ONES8 + 8] = 1.0
    cbf[:SEGS_PER_CORE, CB_ID:CB_ID + SEGS_PER_CORE] = np.eye(
        SEGS_PER_CORE, dtype=np.float32)
    cbf[:, CB_W7:CB_W7 + KS * NCLS] = np.asarray(W7, np.float32).reshape(
        KS, P, NCLS).transpose(1, 0, 2).reshape(P, KS * NCLS)

    rwb = np.zeros((1, 64), dtype=np.float32)
    rwb[0, RW_B7:RW_B7 + NCLS] = np.asarray(b7, np.float32).reshape(-1)
    rwb[0, RW_ONES:RW_ONES + SEGS_PER_CORE] = 1.0

    c8 = np.zeros((1, 2, 1536), dtype=np.float32)
    c8[0, :, 0:P] = 1.0
    b4f = np.asarray(b4, np.float32).reshape(-1)
    b4hi = b4f.astype(E4NP).astype(np.float32)
    b4lo = (b4f - b4hi).astype(E4NP).astype(np.float32)
    c8[0, 0, 512:1536] = b4hi
    c8[0, 1, 512:1536] = b4lo

    shared = dict(
        W1p=W1p.astype(BFNP),
        W2q=dr_pack(W2, E4NP),
        W3q=dr_pack(W3, E4NP),
        W4q=dr_pack(W4, E4NP),
        W5rep=np.broadcast_to(np.asarray(W5, np.float32).reshape(1, HID),
                              (P, HID)).astype(BFNP),
        W6b=dr_pack(W6, BFNP),
        b6r=np.asarray(b6, np.float32).reshape(1, HID).astype(BFNP),
        miscc=misc,
        cbf=cbf.astype(BFNP),
        c8=c8.astype(E4NP),
        rwb=rwb.astype(BFNP),
    )

    in_maps = []
    for core in range(NCORES):
        segs = bins[core]
        xs = [x[starts[s]:starts[s + 1]] for s in segs]
        xcat = np.concatenate(xs, axis=0) if xs else np.zeros((0, FEAT), np.float32)
        n = xcat.shape[0]
        xT = np.zeros((P, m_pad), dtype=np.float32)
        xT[:FEAT, :n] = xcat.T
        xT[FEAT, :n] = 1.0  # constant feature -> b1
        A = np.zeros((m_pad, SEGS_PER_CORE), dtype=np.float32)
        off = 0
        for j, s in enumerate(segs):
            ln = int(counts[s])
            A[off:off + ln, j] = 1.0
            off += ln
        im = dict(shared)
        im["xT"] = xT.astype(BFNP)
        # partition-major layout [P, frt, S]: Ah[p, t, s] = A[t*128 + p, s]
        im["Amat"] = np.ascontiguousarray(
            A.reshape(frt, P, SEGS_PER_CORE).transpose(1, 0, 2)).astype(BFNP)
        in_maps.append(im)
    return in_maps, bins, m_pad


_PROGRAM_CACHE: dict[int, object] = {}


def kernel(**inputs) -> np.ndarray:
    in_maps, bins, m_pad = prepare_inputs(**inputs)
    nc = _PROGRAM_CACHE.get(m_pad)
    if nc is None:
        nc = _build_program(m_pad)
        _PROGRAM_CACHE[m_pad] = nc
    res = run_bass_kernel_spmd(nc, in_maps, core_ids=list(range(NCORES)))
    out = np.zeros((NSEG, NCLS), dtype=np.float32)
    for core in range(NCORES):
        out[bins[core]] = res.results[core]["out"]
    return out


# revision 15
# speedup vs baseline: 2.7382x; 1.3492x over previous
"""Trainium2 Bass kernel for nn_Dnn_with_Attention (ragged attention-pooled DNN).

Contract: kernel(**inputs) takes FULL unsharded numpy inputs (keys as in
reference.setup_inputs()) and returns the FULL [256, 10] float32 output.

Strategy (data-parallel over utterances, 8 NeuronCores):
  - Host: greedily balance the 256 segments over 8 cores (32 whole segments
    each), gather each core's frames, transpose x to feature-major
    bf16 [128(feat-padded), M_PAD] and build a per-frame one-hot segment
    membership matrix A (bf16).  A row of ones is appended as feature 78 so
    b1 folds into W1.
  - Device (per core): L1 in bf16 (feature-major, [1024, frames]); L2/L3/L4
    run in fp8 e4m3 with MatmulPerfMode.DoubleRow (two 128-K slices per
    instruction at 0.5 cycles/row, ~4x the f32r rate).  Weights W2/W3/W4 are
    host-quantized to e4m3; inter-layer activations are written as e4m3
    directly by the relu ops.  L4 produces frame-major h4 in bf16; b4 is
    added via a DoubleRow matmul against a host-packed (hi, lo) e4m3 pair so
    the quantization error cancels.  Scores use a single fused DVE
    tensor_tensor_reduce (h4 * W5 -> per-frame sum) in bf16 2x mode;
    e = max(exp(score + b5), 1) folds the relu.  Segment softmax pooling is
    small PE matmuls E.T @ h4 (E = A * e, bf16) accumulated into persistent
    PSUM across all chunks; the denominator comes from E.T @ ones into the
    same PSUM bank at a different partition quadrant.  The final
    per-utterance MLP runs once at the end in bf16.
  - Per-layer relu work is spread across three engines so the PE stays the
    bottleneck: L1 on GpSimd, L2/L4 on Scalar (activation), L3 on DVE
    (fused add+max tensor_scalar).
  - The whole program is emitted statically (no hardware loop).
"""

import sys

sys.path.insert(0, "/opt/trn_rl_repo")

import numpy as np
import ml_dtypes

import concourse.bass as bass
import concourse.mybir as mybir
import concourse.tile as tile
from concourse import bacc
from concourse.bass_utils import run_bass_kernel_spmd

P = 128
FEAT = 78
HID = 1024
NCLS = 10
NSEG = 256
NCORES = 8
SEGS_PER_CORE = NSEG // NCORES
CH = 512           # frames per chunk (free dim of the layer matmuls)
FRT_PER_CH = CH // P
KS = HID // P      # 8 k-subtiles
F32 = mybir.dt.float32
F32R = mybir.dt.float32r
BF16 = mybir.dt.bfloat16
F8 = mybir.dt.float8e4
DR = mybir.MatmulPerfMode.DoubleRow
E4NP = ml_dtypes.float8_e4m3
BFNP = ml_dtypes.bfloat16

# misc constant tile column layout ([128, 32] f32, host-packed)
MC_B2 = 0          # cols 0..7   : b2 striped [128, 8]
MC_B3 = 8          # cols 8..15  : b3 striped
MC_B5 = 17         # col 17      : b5 replicated down partitions
# bf16 const tile ([128, 96])
CB_ONES8 = 0       # cols 0..7  : ones (denom matmul rhs)
CB_ID = 8          # cols 8..39, rows 0..31: 32x32 identity
CB_W7 = 40         # cols 40..119?? keep within 96: W7 as [128, 8, 10] -> 80 cols
# fp8 const row ([1, 2, 1536]): ones pair + b4 (hi, lo) pair
# row layout [1, 2, 1536]: [:, :, 0:128] ones, [:, :, 512:1536] b4 hi/lo
# simpler: two fields side by side, see prepare_inputs
# bf16 row consts ([1, 64])
RW_B7 = 0          # cols 0..9 : b7
RW_ONES = 16       # cols 16..48 : ones row (bias matmuls, final MLP)


def _segment_ids(lengths: np.ndarray, total: int) -> np.ndarray:
    """Replicate jnp.repeat(arange(n), lengths, total_repeat_length=total)."""
    lengths = np.asarray(lengths, dtype=np.int64)
    seg = np.repeat(np.arange(lengths.shape[0], dtype=np.int32), np.maximum(lengths, 0))
    if seg.shape[0] >= total:
        return seg[:total]
    pad_val = seg[-1] if seg.shape[0] > 0 else np.int32(0)
    return np.concatenate([seg, np.full(total - seg.shape[0], pad_val, np.int32)])


def _balance_segments(lengths: np.ndarray) -> list[list[int]]:
    """Assign 256 segments to 8 cores, 32 each, minimizing max frame count."""
    order = np.argsort(-lengths, kind="stable")
    loads = [0] * NCORES
    bins: list[list[int]] = [[] for _ in range(NCORES)]
    for s in order:
        cands = [c for c in range(NCORES) if len(bins[c]) < SEGS_PER_CORE]
        c = min(cands, key=lambda c: (loads[c], c))
        bins[c].append(int(s))
        loads[c] += int(lengths[s])
    for b in bins:
        b.sort()
    return bins


def _build_program(m_pad: int):
    """Emit the Bass/Tile program for one core with m_pad frames (static)."""
    nch = m_pad // CH
    frt = m_pad // P
    S = SEGS_PER_CORE

    nc = bacc.Bacc("TRN2", target_bir_lowering=False, debug=False,
                   num_devices=NCORES)

    xT_d = nc.dram_tensor("xT", [P, m_pad], BF16, kind="ExternalInput")
    A_d = nc.dram_tensor("Amat", [P, frt, S], BF16, kind="ExternalInput")
    W1_d = nc.dram_tensor("W1p", [P, HID], BF16, kind="ExternalInput")
    W2_d = nc.dram_tensor("W2q", [P, KS, HID], F8, kind="ExternalInput")
    W3_d = nc.dram_tensor("W3q", [P, KS, HID], F8, kind="ExternalInput")
    W4_d = nc.dram_tensor("W4q", [P, KS, HID], F8, kind="ExternalInput")
    W5_d = nc.dram_tensor("W5rep", [P, HID], BF16, kind="ExternalInput")
    W6_d = nc.dram_tensor("W6b", [P, KS, HID], BF16, kind="ExternalInput")
    b6_d = nc.dram_tensor("b6r", [1, HID], BF16, kind="ExternalInput")
    misc_d = nc.dram_tensor("miscc", [P, 32], F32, kind="ExternalInput")
    cbf_d = nc.dram_tensor("cbf", [P, 128], BF16, kind="ExternalInput")
    c8_d = nc.dram_tensor("c8", [1, 2, 1536], F8, kind="ExternalInput")
    rw_d = nc.dram_tensor("rwb", [1, 64], BF16, kind="ExternalInput")
    out_d = nc.dram_tensor("out", [S, NCLS], F32, kind="ExternalOutput")

    RELU = mybir.ActivationFunctionType.Relu
    EXP = mybir.ActivationFunctionType.Exp
    MULT = mybir.AluOpType.mult
    ADD = mybir.AluOpType.add
    MAX = mybir.AluOpType.max

    with tile.TileContext(nc) as tc:
        with (
            tc.tile_pool(name="wpool", bufs=1) as wpool,
            tc.tile_pool(name="xpool", bufs=2) as xpool,
            tc.tile_pool(name="apool", bufs=6) as apool,
            tc.tile_pool(name="h1pool", bufs=2) as h1pool,
            tc.tile_pool(name="h2pool", bufs=2) as h2pool,
            tc.tile_pool(name="h3pool", bufs=2) as h3pool,
            tc.tile_pool(name="h4pool", bufs=3) as h4pool,
            tc.tile_pool(name="scrpool", bufs=2) as scrpool,
            tc.tile_pool(name="colpool", bufs=2) as colpool,
            tc.tile_pool(name="epool", bufs=3) as epool,
            tc.tile_pool(name="fpool", bufs=1) as fpool,
            tc.tile_pool(name="psL", bufs=3, space="PSUM") as psL,
            tc.tile_pool(name="psAcc", bufs=1, space="PSUM") as psAcc,
        ):
            # ---- resident constants/weights ----
            W1s = wpool.tile([P, HID], BF16, tag="W1")
            nc.sync.dma_start(W1s[:], W1_d.ap())

            def load_w(d, tagp, dt):
                t = wpool.tile([P, KS, HID], dt, tag=tagp)
                for k in range(KS):
                    nc.sync.dma_start(t[:, k, :], d.ap()[:, k, :])
                return t

            W2s = load_w(W2_d, "W2q", F8)
            W3s = load_w(W3_d, "W3q", F8)
            W4s = load_w(W4_d, "W4q", F8)
            W5s4 = wpool.tile([P, FRT_PER_CH, HID], BF16, tag="W5")
            for f in range(FRT_PER_CH):
                nc.sync.dma_start(W5s4[:, f, :], W5_d.ap())
            misc = wpool.tile([P, 32], F32, tag="misc")
            nc.sync.dma_start(misc[:], misc_d.ap())
            cbf = wpool.tile([P, 128], BF16, tag="cbf")
            nc.sync.dma_start(cbf[:], cbf_d.ap())
            c8 = wpool.tile([1, 2, 1536], F8, tag="c8")
            nc.sync.dma_start(c8[:], c8_d.ap())
            rwb = wpool.tile([1, 64], BF16, tag="rwb")
            nc.sync.dma_start(rwb[:], rw_d.ap())

            b5col = misc[:, MC_B5:MC_B5 + 1]
            ones8 = cbf[:, CB_ONES8:CB_ONES8 + 8]
            ident = cbf[:S, CB_ID:CB_ID + S]
            W7v = cbf[:, CB_W7:CB_W7 + KS * NCLS].rearrange(
                "p (o c) -> p o c", c=NCLS)
            b7row = rwb[:, RW_B7:RW_B7 + NCLS]
            ones_row = rwb[:, RW_ONES:RW_ONES + S]
            ones_pair8 = c8[:, :, 0:P]          # [1, 2, 128] of ones (fp8)
            b4pair = c8[:, :, 512:1536]         # [1, 2, 1024] b4 (hi, lo)

            # persistent PSUM accumulators:
            #   bank0: pooled0 [0:32, 0:512], denom [32:64, 0:8]
            #   bank1: pooled1 [0:32, 0:512]
            acc0 = psAcc.tile([P, 512], F32, tag="acc0")
            acc1 = psAcc.tile([P, 512], F32, tag="acc1")
            pooled0 = acc0[0:S, :]
            pooled1 = acc1[0:S, :]
            denom = acc0[S:2 * S, 0:8]

            # ---- main pass: 5-stage software pipeline over chunks ----
            # Stage k of chunk c runs in iteration c+k, so every cross-engine
            # dependency (matmul -> relu -> next layer's matmul) has a full
            # iteration (~15us) of slack and the PE never waits on the relus.
            st_ = {}   # per-chunk tile state

            def relu_ps(out, in_, bias, eng):
                """relu(in_ + bias) -> out (fp8/bf16 cast) on Scalar or DVE."""
                if eng == 0:
                    nc.scalar.activation(out, in_, RELU,
                                         bias=0.0 if bias is None else bias)
                elif bias is None:
                    nc.vector.tensor_scalar_max(out, in_, 0.0)
                else:
                    nc.vector.tensor_scalar(out=out, in0=in_, scalar1=bias,
                                            scalar2=0.0, op0=ADD, op1=MAX)

            def s1(c):  # DMA + L1 (bf16) -> h1 fp8 (batched relu, alt eng)
                xt = xpool.tile([P, CH], BF16, tag="x")
                nc.sync.dma_start(xt[:], xT_d.ap()[:, c * CH:(c + 1) * CH])
                ag = apool.tile([P, FRT_PER_CH, S], BF16, tag="A")
                nc.sync.dma_start(
                    ag[:], A_d.ap()[:, c * FRT_PER_CH:(c + 1) * FRT_PER_CH, :])
                h1 = h1pool.tile([P, KS, CH], F8, tag="h1")
                for j in range(KS // 2):
                    ps = psL.tile([P, 2 * CH], F32, tag="mm")
                    for i in range(2):
                        m = 2 * j + i
                        nc.tensor.matmul(ps[:, i * CH:(i + 1) * CH],
                                         W1s[:, m * P:(m + 1) * P], xt[:],
                                         start=True, stop=True)
                    relu_ps(h1[:, 2 * j:2 * j + 2, :], ps[:], None, j % 2)
                st_[c] = {"ag": ag, "h1": h1}

            def _mid_layer(c, Ws, hin_key, hout_key, pool, boff, flip):
                """L2/L3: fp8 DoubleRow + per-m relu(+bias), alternating
                engines per psum-tile half so the drain keeps up with PE."""
                hin = st_[c][hin_key]
                hout = pool.tile([P, KS, CH], F8, tag=hout_key)
                for j in range(KS // 2):
                    ps = psL.tile([P, 2 * CH], F32, tag="mm")
                    for i in range(2):
                        m = 2 * j + i
                        for t in range(KS // 2):
                            nc.tensor.matmul(
                                ps[:, i * CH:(i + 1) * CH],
                                Ws[:, 2 * t:2 * t + 2, m * P:(m + 1) * P],
                                hin[:, 2 * t:2 * t + 2, :],
                                start=(t == 0), stop=(t == KS // 2 - 1),
                                perf_mode=DR)
                        relu_ps(hout[:, m, :], ps[:, i * CH:(i + 1) * CH],
                                misc[:, boff + m:boff + m + 1], (m + flip) % 2)
                st_[c][hout_key] = hout

            def s2(c):
                _mid_layer(c, W2s, "h1", "h2", h2pool, MC_B2, 0)

            def s3(c):
                _mid_layer(c, W3s, "h2", "h3", h3pool, MC_B3, 1)

            def s4(c):  # L4 fp8 DoubleRow -> h4 bf16; scores -> E (bf16)
                h3 = st_[c]["h3"]
                ag = st_[c]["ag"]
                h4 = h4pool.tile([P, FRT_PER_CH, HID], BF16, tag="h4")
                for f in range(FRT_PER_CH):
                    ps4 = psL.tile([P, 2 * CH], F32, tag="mm")
                    for n in range(2):
                        o = ps4[:, n * 512:(n + 1) * 512]
                        nc.tensor.matmul(o, ones_pair8,
                                         b4pair[:, :, n * 512:(n + 1) * 512],
                                         start=True, stop=False, perf_mode=DR)
                        for t in range(KS // 2):
                            nc.tensor.matmul(
                                o, h3[:, 2 * t:2 * t + 2, f * P:(f + 1) * P],
                                W4s[:, 2 * t:2 * t + 2, n * 512:(n + 1) * 512],
                                start=False, stop=(t == KS // 2 - 1),
                                perf_mode=DR)
                    relu_ps(h4[:, f, :], ps4[:], None, f % 2)
                # scores, fully batched: one GpSimd product over all four
                # f-tiles, one DVE reduce (innermost axis), one exp, one
                # clamp; per-f E columns on GpSimd.  E is consumed by the
                # pooling matmuls one iteration later.
                scr = scrpool.tile([P, FRT_PER_CH, HID], BF16, tag="scr")
                for f in range(FRT_PER_CH):
                    nc.gpsimd.tensor_mul(scr[:, f, :], h4[:, f, :],
                                         W5s4[:, f, :])
                st_[c]["h4"] = h4
                st_[c]["scr"] = scr

            def s4b(c):  # score reduce/exp/clamp/E, one iteration after the
                # products, so none of it head-of-line-blocks the act queues.
                ag = st_[c]["ag"]
                scr = st_[c]["scr"]
                ct = colpool.tile([P, FRT_PER_CH], F32, tag="ctb")
                for f in range(FRT_PER_CH):
                    nc.vector.tensor_reduce(out=ct[:, f:f + 1],
                                            in_=scr[:, f, :],
                                            axis=mybir.AxisListType.X, op=ADD)
                etg = epool.tile([P, FRT_PER_CH, S], BF16, tag="E")
                ec = colpool.tile([P, 2 * FRT_PER_CH], F32, tag="ec")
                nc.scalar.activation(ec[:, :FRT_PER_CH], ct[:], EXP,
                                     bias=b5col)
                nc.gpsimd.tensor_scalar_max(ec[:, FRT_PER_CH:],
                                            ec[:, :FRT_PER_CH], 1.0)
                for f in range(FRT_PER_CH):
                    nc.gpsimd.tensor_scalar_mul(
                        etg[:, f, :], ag[:, f, :],
                        ec[:, FRT_PER_CH + f:FRT_PER_CH + f + 1])
                st_[c]["et"] = etg

            def s5(c):  # pooling matmuls (persistent PSUM accumulation)
                h4 = st_[c]["h4"]
                etg = st_[c]["et"]
                first = c == 0
                last = c == nch - 1
                for f in range(FRT_PER_CH):
                    et = etg[:, f, :]
                    st = bool(first and f == 0)
                    sp = bool(last and f == FRT_PER_CH - 1)
                    # pooled0/denom share a PSUM bank at different partition
                    # quadrants; the sim's group check is partition-blind so
                    # it must be skipped (values verified exact in CoreSim).
                    nc.tensor.matmul(pooled0, et, h4[:, f, :512],
                                     start=st, stop=sp, skip_group_check=True)
                    nc.tensor.matmul(pooled1, et, h4[:, f, 512:],
                                     start=st, stop=sp, skip_group_check=True)
                    nc.tensor.matmul(denom, et, ones8,
                                     start=st, stop=sp, skip_group_check=True)
                del st_[c]

            sched = ((s1, 0), (s2, 1), (s3, 2), (s4, 3), (s4b, 4), (s5, 5))
            for i in range(nch + 5):
                for stage, off in sched:
                    c = i - off
                    if 0 <= c < nch:
                        stage(c)

            # ---- final per-utterance MLP ----
            W6s = load_w(W6_d, "W6b", BF16)
            b6s = wpool.tile([1, HID], BF16, tag="b6")
            nc.sync.dma_start(b6s[:], b6_d.ap())

            # 1/denom: copy the [32:64] psum quadrant to SBUF, then DMA-shift
            # it down to partitions 0:32 (engines can't move across lanes)
            dtmp = fpool.tile([2 * S, 1], F32, tag="dtmp")
            nc.vector.tensor_copy(out=dtmp[S:2 * S, 0:1], in_=denom[:, 0:1])
            fc = colpool.tile([S, 4], F32, tag="col")
            nc.sync.dma_start(fc[:, 0:1], dtmp[S:2 * S, 0:1])
            nc.vector.reciprocal(fc[:, 1:2], fc[:, 0:1])

            # pooled (normalized) in f32 for the PE transpose
            pooled_sb = fpool.tile([S, HID], F32, tag="pooled")
            nc.vector.tensor_scalar_mul(pooled_sb[:, :512], pooled0,
                                        fc[:, 1:2])
            nc.vector.tensor_scalar_mul(pooled_sb[:, 512:], pooled1,
                                        fc[:, 1:2])

            # transpose pooled -> pooledT [hid, seg] (bf16 via cast copies)
            identf = fpool.tile([S, S], F32, tag="identf")
            nc.vector.tensor_copy(out=identf[:], in_=ident)
            tposed = fpool.tile([P, KS, 2 * S], BF16, tag="tposed")
            pooledT = tposed[:, :, :S]
            gT = tposed[:, :, S:]
            for k in range(KS):
                pst = psL.tile([P, 2 * CH], F32, tag="mm")
                nc.tensor.transpose(pst[:, :S], pooled_sb[:, k * P:(k + 1) * P],
                                    identf[:])
                nc.vector.tensor_copy(out=pooledT[:, k, :], in_=pst[:, :S])

            # g = relu(pooled @ W6 + b6)   (seg-major [S, HID], bf16)
            g_sb = fpool.tile([S, HID], BF16, tag="g")
            for n in range(2):
                psg = psL.tile([P, 2 * CH], F32, tag="mm")
                for k in range(KS):
                    nc.tensor.matmul(psg[:S, :512], pooledT[:, k, :],
                                     W6s[:, k, n * 512:(n + 1) * 512],
                                     start=(k == 0), stop=False)
                nc.tensor.matmul(psg[:S, :512], ones_row,
                                 b6s[:, n * 512:(n + 1) * 512],
                                 start=False, stop=True)
                nc.scalar.activation(g_sb[:, n * 512:(n + 1) * 512],
                                     psg[:S, :512], RELU)

            # gT [hid, seg] (transpose back via f32 staging)
            gf = fpool.tile([S, HID], F32, tag="gf")
            nc.vector.tensor_copy(out=gf[:], in_=g_sb[:])
            for k in range(KS):
                pst = psL.tile([P, 2 * CH], F32, tag="mm")
                nc.tensor.transpose(pst[:, :S], gf[:, k * P:(k + 1) * P],
                                    identf[:])
                nc.vector.tensor_copy(out=gT[:, k, :], in_=pst[:, :S])

            # out = g @ W7 + b7
            pso = psL.tile([P, 2 * CH], F32, tag="mm")
            for k in range(KS):
                nc.tensor.matmul(pso[:S, :NCLS], gT[:, k, :], W7v[:, k, :],
                                 start=(k == 0), stop=False)
            nc.tensor.matmul(pso[:S, :NCLS], ones_row, b7row,
                             start=False, stop=True)
            oc = colpool.tile([S, 16], F32, tag="oc")
            nc.vector.tensor_copy(out=oc[:, :NCLS], in_=pso[:S, :NCLS])
            nc.sync.dma_start(out_d.ap()[:], oc[:, :NCLS])

    nc.compile()
    return nc


def prepare_inputs(x, W1, b1, W2, b2, W3, b3, W4, b4, W5, b5, W6, b6, W7, b7,
                   lengths):
    """Host-side sharding/packing. Returns (in_maps, bins, m_pad)."""
    x = np.ascontiguousarray(np.asarray(x, dtype=np.float32))
    lengths = np.asarray(lengths)
    total = x.shape[0]
    seg_ids = _segment_ids(lengths, total)
    counts = np.bincount(seg_ids, minlength=NSEG).astype(np.int64)
    starts = np.zeros(NSEG + 1, dtype=np.int64)
    starts[1:] = np.cumsum(counts)

    bins = _balance_segments(counts)
    core_frames = [int(sum(counts[s] for s in b)) for b in bins]
    m_pad = ((max(core_frames) + CH - 1) // CH) * CH
    frt = m_pad // P

    W1p = np.zeros((P, HID), dtype=np.float32)
    W1p[:FEAT] = np.asarray(W1, dtype=np.float32)
    W1p[FEAT] = np.asarray(b1, dtype=np.float32)

    def dr_pack(W, dt):
        """[1024, 1024] -> [128, 8, 1024] with Wq[p, k, m] = W[k*128+p, m]."""
        Wf = np.asarray(W, np.float32).reshape(KS, P, HID)
        return np.ascontiguousarray(Wf.transpose(1, 0, 2)).astype(dt)

    misc = np.zeros((P, 32), dtype=np.float32)
    misc[:, MC_B2:MC_B2 + KS] = np.asarray(b2, np.float32).reshape(KS, P).T
    misc[:, MC_B3:MC_B3 + KS] = np.asarray(b3, np.float32).reshape(KS, P).T
    misc[:, MC_B5] = np.float32(np.asarray(b5, np.float32).reshape(-1)[0])

    cbf = np.zeros((P, 128), dtype=np.float32)
    cbf[:, CB_ONES8:CB_ONES8 + 8] = 1.0
    cbf[:SEGS_PER_CORE, CB_ID:CB_ID + SEGS_PER_CORE] = np.eye(
        SEGS_PER_CORE, dtype=np.float32)
    cbf[:, CB_W7:CB_W7 + KS * NCLS] = np.asarray(W7, np.float32).reshape(
        KS, P, NCLS).transpose(1, 0, 2).reshape(P, KS * NCLS)

    rwb = np.zeros((1, 64), dtype=np.float32)
    rwb[0, RW_B7:RW_B7 + NCLS] = np.asarray(b7, np.float32).reshape(-1)
    rwb[0, RW_ONES:RW_ONES + SEGS_PER_CORE] = 1.0

    c8 = np.zeros((1, 2, 1536), dtype=np.float32)
    c8[0, :, 0:P] = 1.0
    b4f = np.asarray(b4, np.float32).reshape(-1)
    b4hi = b4f.astype(E4NP).astype(np.float32)
    b4lo = (b4f - b4hi).astype(E4NP).astype(np.float32)
    c8[0, 0, 512:1536] = b4hi
    c8[0, 1, 512:1536] = b4lo

    shared = dict(
        W1p=W1p.astype(BFNP),
        W2q=dr_pack(W2, E4NP),
        W3q=dr_pack(W3, E4NP),
        W4q=dr_pack(W4, E4NP),
        W5rep=np.broadcast_to(np.asarray(W5, np.float32).reshape(1, HID),
                              (P, HID)).astype(BFNP),
        W6b=dr_pack(W6, BFNP),
        b6r=np.asarray(b6, np.float32).reshape(1, HID).astype(BFNP),
        miscc=misc,
        cbf=cbf.astype(BFNP),
        c8=c8.astype(E4NP),
        rwb=rwb.astype(BFNP),
    )

    in_maps = []
    for core in range(NCORES):
        segs = bins[core]
        xs = [x[starts[s]:starts[s + 1]] for s in segs]
        xcat = np.concatenate(xs, axis=0) if xs else np.zeros((0, FEAT), np.float32)
        n = xcat.shape[0]
        xT = np.zeros((P, m_pad), dtype=np.float32)
        xT[:FEAT, :n] = xcat.T
        xT[FEAT, :n] = 1.0  # constant feature -> b1
        A = np.zeros((m_pad, SEGS_PER_CORE), dtype=np.float32)
        off = 0
        for j, s in enumerate(segs):
            ln = int(counts[s])
            A[off:off + ln, j] = 1.0
            off += ln
        im = dict(shared)
        im["xT"] = xT.astype(BFNP)
        # partition-major layout [P, frt, S]: Ah[p, t, s] = A[t*128 + p, s]
        im["Amat"] = np.ascontiguousarray(
            A.reshape(frt, P, SEGS_PER_CORE).transpose(1, 0, 2)).astype(BFNP)
        in_maps.append(im)
    return in_maps, bins, m_pad


_PROGRAM_CACHE: dict[int, object] = {}


def kernel(**inputs) -> np.ndarray:
    in_maps, bins, m_pad = prepare_inputs(**inputs)
    nc = _PROGRAM_CACHE.get(m_pad)
    if nc is None:
        nc = _build_program(m_pad)
        _PROGRAM_CACHE[m_pad] = nc
    res = run_bass_kernel_spmd(nc, in_maps, core_ids=list(range(NCORES)))
    out = np.zeros((NSEG, NCLS), dtype=np.float32)
    for core in range(NCORES):
        out[bins[core]] = res.results[core]["out"]
    return out
